# revision 11
# baseline (speedup 1.0000x reference)
"""Trainium2 Bass kernel for DifferentiableRGBtoVel (soft-nearest-neighbor
colormap inversion).

velocity(p) = sum_k v_k e^{-100 d_k(p)} / sum_k e^{-100 d_k(p)},
d_k(p) = |p - c_k|^2.

The softmax shift uses the linear surrogate B_p = 100*sum_c(p_c) - 37.5 of
100|p|^2 (minimax-optimal linear fit of x^2 on [0,1]; the shift cancels
exactly in the num/den ratio, so only fp32-range safety matters).

Per-core pipeline in [k, pix] layout (partition = colormap index), super-tiles
of 1024 pixels ([128, 2048] PSUM = [A0|B0|A1|B1]):
  scores: float32r matmuls at full PE rate (1 cyc/col).  float32r truncates
          operands to 11 mantissa bits, so operands are Veltkamp-split into
          hi (11-bit, exact in f32r) + lo parts host-side; one K=11 matmul
          per 512-px chunk per 128-color half reproduces fp32-exact scores:
          rows = [chi*phi(3), chi*plo(3), clo*phi(3), bhi*1, blo*1].
  exp:    ONE ACT instruction per super-tile (FD=2048), func=Exp, scale=200.
  num/den: fp32 [128,2] matmuls (cols = [1, v_k]) contracting exp through the
          PE; fp32 runs at 1/4 rate, so chains rotate over the 4 PE column
          groups (tile_position=(0,32m)) to run 4 chains concurrently.
          Results land in a corner of the (already-consumed) score PSUM tile.
  divide: DVE copy -> SBUF, partition-compacting DMA into dense [128,512]
          accumulators (den rows 0-63, num rows 64-127), DVE reciprocal +
          multiply per 32-super-tile group, one output DMA per group.
"""

import numpy as np

import concourse.bass as bass
import concourse.mybir as mybir
import concourse.tile as tile_mod
from concourse.tile import TileContext
from concourse.vector_clock import ScopedClock, VectorClock
from concourse.bass_utils import run_bass_kernel_spmd

# ---------------------------------------------------------------- constants
N_CORES = 8
NB, C, H, W = 4, 3, 512, 512
K = 256
KH = 128                       # colors per half
PIX_PER_CORE = NB * H * W // N_CORES   # 131072
CHUNK = 512                    # pixels per matmul (4-byte rhs max N)
TILE_PIX = 1024                # pixels per super-tile ([128, 2048] psum)
GROUP = 32                     # super-tiles per division group
IMG_BATCH = 8                  # super-tiles per image DMA
KC = 11                        # score-matmul contraction rows

_FP32 = mybir.dt.float32
_F32R = mybir.dt.float32r


# ------------------------------------------------- walrus sync-wait limits
# The walrus build in this container rejects instructions carrying more than
# one sem wait ("Too many sync wait commands").  Tile emits several; split
# them onto same-engine NoOps.
def _split_drain_and_barrier(self, tick_clock, wait_clock):
    nc = self.nc
    vec = list(tick_clock.global_clock)
    for i, v in enumerate(vec):
        if v > 0:
            w = [0] * len(vec)
            w[i] = v
            inst = nc.sync.nop(nofuse=True, hint="split_drain_wait")
            wait_clock.add_sem_waits(inst.ins, ScopedClock({None: VectorClock(w)}))
    nc.sync.drain()
    nc.all_engine_barrier()
    assert self.sems is not None
    popped = nc._tile_sem_poison_stack.pop()
    assert popped is self._sem_poison
    nc.clear_and_free_semaphores(list(self.sems.allocated().values()))
    nc.all_engine_barrier()


tile_mod.TileContext._drain_and_barrier = _split_drain_and_barrier

MAX_WAITS = 1


def _split_excess_waits(nc, maxw=MAX_WAITS):
    for f in nc.m.functions:
        for bb in f.blocks:
            out = []
            for inst in bb.instructions:
                si = inst.sync_info
                if si is not None and len(si.on_wait) > maxw:
                    waits = list(si.on_wait)
                    excess, keep = waits[:-maxw], waits[-maxw:]
                    for i in range(0, len(excess), maxw):
                        nop = mybir.InstNoOp(
                            name=nc.get_next_instruction_name(),
                            sync_info=mybir.SyncInfo(
                                on_wait=excess[i:i + maxw], on_update=[]),
                            bass_nofuse=True,
                            engine=inst.engine,
                        )
                        out.append(nop)
                    inst.sync_info = mybir.SyncInfo(
                        on_wait=keep, on_update=list(si.on_update))
                out.append(inst)
            bb.instructions = out


# ------------------------------------------------------------- bass builder
def build_kernel(pix_per_core: int = PIX_PER_CORE):
    n_tiles = pix_per_core // TILE_PIX
    n_groups = (n_tiles + GROUP - 1) // GROUP

    nc = bass.Bass(trn_type="TRN2", name="rgb2vel")
    imgD = nc.dram_tensor("img", [KC, pix_per_core], _FP32, kind="ExternalInput")
    cmD = nc.dram_tensor("cmt", [KC, K], _FP32, kind="ExternalInput")
    vmD = nc.dram_tensor("vmat", [KH, 4], _FP32, kind="ExternalInput")
    velD = nc.dram_tensor("vel", [pix_per_core // 512, 512], _FP32,
                          kind="ExternalOutput")

    ExpF = mybir.ActivationFunctionType.Exp

    with TileContext(nc) as tc:
        with (
            tc.tile_pool(name="const", bufs=1) as cpool,
            tc.tile_pool(name="img", bufs=3) as ipool,
            tc.tile_pool(name="exp", bufs=3) as epool,
            tc.tile_pool(name="stg", bufs=4) as stgpool,
            tc.tile_pool(name="acc", bufs=2) as accpool,
            tc.tile_pool(name="divp", bufs=2) as dpool,
            tc.tile_pool(name="score", bufs=2, space="PSUM") as spool,
        ):
            # persistent constants
            cma = cpool.tile([KC, KH], _F32R, tag="cma")
            nc.sync.dma_start(cma[:], cmD[:, 0:KH].bitcast(_F32R))
            cmb = cpool.tile([KC, KH], _F32R, tag="cmb")
            nc.sync.dma_start(cmb[:], cmD[:, KH:K].bitcast(_F32R))
            vm = cpool.tile([KH, 4], _FP32, tag="vm")
            nc.sync.dma_start(vm[:], vmD[:])

            state = {"img": None}

            def do_tile(t, dnv, j):
                # ---- image slab DMA, batched over IMG_BATCH super-tiles
                if t % IMG_BATCH == 0:
                    imgt = ipool.tile(
                        [KC, IMG_BATCH * TILE_PIX], _F32R, tag="img")
                    sl = slice(t * TILE_PIX, (t + IMG_BATCH) * TILE_PIX)
                    nc.sync.dma_start(imgt[:], imgD[:, sl].bitcast(_F32R))
                    state["img"] = imgt
                img = state["img"]
                ioff = (t % IMG_BATCH) * TILE_PIX

                # ---- scores: [A0|B0|A1|B1] in one [128, 2048] psum tile
                ps = spool.tile([128, 2 * TILE_PIX], _FP32, tag="score")
                for q in range(2):
                    rsl = slice(ioff + q * CHUNK, ioff + (q + 1) * CHUNK)
                    base = 2 * q * CHUNK
                    nc.tensor.matmul(ps[:, base:base + CHUNK], lhsT=cma[:],
                                     rhs=img[:, rsl], start=True, stop=True)
                    nc.tensor.matmul(ps[:, base + CHUNK:base + 2 * CHUNK],
                                     lhsT=cmb[:],
                                     rhs=img[:, rsl], start=True, stop=True)

                # ---- exp: one ACT instruction, FD=2048
                ex = epool.tile([128, 2 * TILE_PIX], _FP32, tag="exp")
                nc.scalar.activation(ex[:], ps[:], ExpF, bias=0.0, scale=200.0)

                # ---- num/den fp32 chains, rotated over PE column groups
                for q in range(2):
                    m = (2 * t + q) % 4
                    tp = (0, 32 * m) if m else None
                    out = ps[32 * m:32 * m + 2, q * CHUNK:(q + 1) * CHUNK]
                    nc.tensor.matmul(
                        out, lhsT=vm[:, 0:2],
                        rhs=ex[:, 2 * q * CHUNK:(2 * q + 1) * CHUNK],
                        start=True, stop=False, tile_position=tp)
                    nc.tensor.matmul(
                        out, lhsT=vm[:, 2:4],
                        rhs=ex[:, (2 * q + 1) * CHUNK:(2 * q + 2) * CHUNK],
                        start=False, stop=True, tile_position=tp)

                # ---- evacuate [2,512] nd per chain: DVE copy, compact DMA
                stg = stgpool.tile([98, CHUNK], _FP32, tag="stg")
                for q in range(2):
                    m = (2 * t + q) % 4
                    nc.vector.tensor_copy(
                        stg[32 * m:32 * m + 2, :],
                        ps[32 * m:32 * m + 2, q * CHUNK:(q + 1) * CHUNK])
                    nc.sync.dma_start(dnv[:, 2 * j + q, :],
                                      stg[32 * m:32 * m + 2, :])

            def do_group(g, gtiles):
                # dn rows 0..63 = den rows, 64..127 = num rows
                dn = accpool.tile([128, 512], _FP32, tag="dn")
                dnv = dn.rearrange("(a p) w -> a p w", a=2)
                for j in range(gtiles):
                    do_tile(g * GROUP + j, dnv, j)
                rows = 2 * gtiles
                nsh = dpool.tile([64, 512], _FP32, tag="nsh")
                nc.sync.dma_start(nsh[0:rows, :], dn[64:64 + rows, :])
                rcp = dpool.tile([64, 512], _FP32, tag="rcp")
                nc.vector.reciprocal(rcp[0:rows, :], dn[0:rows, :])
                vel = dpool.tile([64, 512], _FP32, tag="vel")
                nc.vector.tensor_tensor(
                    vel[0:rows, :], nsh[0:rows, :], rcp[0:rows, :],
                    mybir.AluOpType.mult)
                nc.sync.dma_start(velD[g * 2 * GROUP:g * 2 * GROUP + rows, :],
                                  vel[0:rows, :])

            for g in range(n_groups):
                do_group(g, min(GROUP, n_tiles - g * GROUP))

    _split_excess_waits(nc)
    return nc


# ----------------------------------------------------------- host wrapper
_CACHE = {}


def _get_nc(pix_per_core):
    if pix_per_core not in _CACHE:
        _CACHE[pix_per_core] = build_kernel(pix_per_core)
    return _CACHE[pix_per_core]


def _hi11(x):
    """Round x to 11 explicit mantissa bits (exact under float32r)."""
    x = np.asarray(x, np.float32)
    xi = x.view(np.uint32).astype(np.uint64)
    xi = (xi + np.uint64(0x800)) & np.uint64(0xFFFFF000)
    return xi.astype(np.uint32).view(np.float32)


def _prep_consts(cmap, v_i):
    cmap = np.asarray(cmap, np.float32)
    v_i = np.asarray(v_i, np.float32)
    c2 = np.sum(cmap * cmap, axis=1, dtype=np.float32)
    cs = (cmap.T - np.float32(0.5)).astype(np.float32)        # [3, K]
    chi = _hi11(cs)
    clo = (cs - chi).astype(np.float32)
    b = ((np.float32(37.5) - np.float32(100.0) * c2)
         / np.float32(200.0)).astype(np.float32)
    bhi = _hi11(b)
    blo = (b - bhi).astype(np.float32)
    cmt = np.empty((KC, K), np.float32)
    cmt[0:3] = chi
    cmt[3:6] = chi
    cmt[6:9] = clo
    cmt[9] = bhi
    cmt[10] = blo
    vmat = np.empty((KH, 4), np.float32)
    vmat[:, 0] = 1.0
    vmat[:, 1] = v_i[0:KH]
    vmat[:, 2] = 1.0
    vmat[:, 3] = v_i[KH:K]
    return cmt, vmat


def _prep_image_slab(slab):
    """slab: [3, n] float32 pixels -> [KC, n] split rows
    [phi(3), plo(3), phi(3), 1, 1]."""
    n = slab.shape[1]
    img = np.empty((KC, n), np.float32)
    phi = _hi11(slab)
    img[0:3] = phi
    img[3:6] = slab - phi
    img[6:9] = phi
    img[9] = 1.0
    img[10] = 1.0
    return img


def _kernel_impl(image, cmap, v_i, _trace=False):
    image = np.ascontiguousarray(np.asarray(image, np.float32))
    cmt, vmat = _prep_consts(cmap, v_i)

    rows_per_core = NB * H // N_CORES          # 256 rows of H per core
    in_maps = []
    for i in range(N_CORES):
        n = (i * rows_per_core) // H
        h0 = (i * rows_per_core) % H
        slab = image[n, :, h0:h0 + rows_per_core, :].reshape(3, -1)
        in_maps.append({"img": _prep_image_slab(slab), "cmt": cmt,
                        "vmat": vmat})

    nc = _get_nc(PIX_PER_CORE)
    res = run_bass_kernel_spmd(nc, in_maps, core_ids=list(range(N_CORES)),
                               trace=_trace)
    out = np.empty((NB, H, W), np.float32)
    for i in range(N_CORES):
        n = (i * rows_per_core) // H
        h0 = (i * rows_per_core) % H
        out[n, h0:h0 + rows_per_core, :] = \
            res.results[i]["vel"].reshape(rows_per_core, W)
    return out, res


def kernel(image, cmap, v_i):
    out, _ = _kernel_impl(image, cmap, v_i)
    return out
